# revision 10
# baseline (speedup 1.0000x reference)
"""Trainium2 Bass kernel for nn_Conv2dLayer_14998025797815.

Reference op (StyleGAN-style hyper-modulated upsampling conv):
  x [8,128,128,128] fp32 ; c [8,512] ; weight [64,128,3,3] ; bias [64]
  w_affine [128,512] ; b_affine [128]
    y  = c @ (w_affine/sqrt(512)).T + b_affine
    x *= (1 + tanh(y))[:,:,None,None]
    x  = upfirdn2d(x, outer(f,f), up=2, pad=(3,2))        f = [1,3,3,1]/8, gain 4
    x  = conv2d(x, flip(weight)/sqrt(128*9), VALID)       -> [8,64,256,256]
    x  = clip(lrelu(x + bias, 0.2) * sqrt(2), +-256)

Math: the zero-insert upsample + 4x4 FIR + 3x3 conv compose into one 6x6
kernel G2 = conv_full(FIR*4, flip(w)); polyphase decomposition over output
parity (a,b) gives four 3x3 convs on the ORIGINAL 128x128 grid:
    out[n,oc,2i+a,2j+b] = sum_{ic,dy,dx} xpad[n,ic,i+dy,j+dx] * G2[oc,ic,1-a+2dy,1-b+2dx]
The per-sample channel scale s = 1+tanh(affine(c)) is folded into the conv
weights on the host. The +-256 clamp is omitted: |out| <= ~6 here.

Mixed precision: the separable FIR makes tap energies asymmetric. Per
matmul group (column parity b), the per-tap share of output energy is
  dx weak col (3 taps): 0.61% + 2.11% + 0.61%, plus two 6.11% taps at
  dy edges of the other cols; the rest carry >11% each.
The 4 weakest taps per group (9.4% of energy) are computed in fp8 e4m3
with DoubleRow perf mode - 2 taps (K=256) per PE pass at the cost of one
bf16 pass - shrinking 9 passes to 7 (5 bf16 + 2 fp8 pairs). Measured
end-to-end rel err stays ~1e-2 vs the 2e-2 budget (quantization error
scales as sqrt(energy share)). fp8 taps read a zero-PADDED fp8 x copy so
k-tile pairs never need edge clipping; bf16 taps read the unpadded bf16
x and clip edge matmuls to the valid region (clipped elements are
exactly the zero-pad contributions). Weights scale 2^K8 keeps fp8
weights in e4m3's normal range (x scaled 2^-K8 to compensate).

Mapping to 8 NeuronCores: data-parallel over batch (one image per core).
Per core: ic on SBUF partitions; two matmul groups by column parity b,
each M = 128 = (a in {0,1}) x (oc in 0..63); PSUM accumulation per
4-row bank; one Prelu ACT per (band, group) fuses bias + leaky-relu +
sqrt2 gain and writes the b-parity interleave in bf16; output DMA
scatters [p=a*64+oc, i, w] rows into NCHW DRAM on the scalar (a=0) and
sync (a=1) rings (bf16, widened to fp32 on the host).
"""
import sys

if '/opt/trn_rl_repo' not in sys.path:
    sys.path.insert(0, '/opt/trn_rl_repo')

from contextlib import ExitStack

import numpy as np

import concourse.bass as bass
import concourse.tile as tile
from concourse import bacc, mybir
from concourse.bass_utils import run_bass_kernel_spmd

N_CORES = 8
IC, OC, H, W = 128, 64, 128, 128
CD = 512
SQ2 = float(np.sqrt(2.0))
ALPHA = 0.2

BAND = 8                      # output phase-grid rows per band
NB = H // BAND                # 16 bands
N_WARM = 48                   # dep-free matmuls: PE p-state ramp; DMA queues
                              # only start after the ~8us engine preamble, so
                              # warmup also covers the first x/w arrival
K8 = 3                        # fp8 scale split: w*2^K8, x*2^-K8

BF16 = mybir.dt.bfloat16
F8 = mybir.dt.float8e4
NP_BF16 = mybir.dt.np(BF16)
NP_F8 = mybir.dt.np(F8)

X_CHUNKS = [(0, 12), (12, 26), (26, 56), (56, 128)]   # bf16 x load (rows)
X8_CHUNKS = [(0, 20), (20, 130)]              # PRE-PADDED fp8 x load (rows)

# 15 8-row bands + two 4-row bands: a small final band shrinks the tail
# (last ACT + last output DMA are half-size)
BANDS = [(i * BAND, BAND) for i in range(NB - 1)] + \
        [(H - BAND, 4), (H - 4, 4)]

# weak taps per group, as DoubleRow pairs; the rest in bf16
PAIRS = {0: [((0, 2), (2, 2)), ((1, 2), (2, 0))],
         1: [((0, 0), (2, 0)), ((1, 0), (2, 2))]}
STRONG = {g: [t for t in [(dy, dx) for dy in range(3) for dx in range(3)]
              if t not in {t_ for pr in PAIRS[g] for t_ in pr}]
          for g in (0, 1)}    # 5 taps per group

_NC = None                    # cached compiled Bass program


def _polyphase_g2(weight: np.ndarray) -> np.ndarray:
    """[64,128,3,3] conv weight -> combined 6x6 polyphase kernel G2 (fp64)."""
    f1 = np.array([1.0, 3.0, 3.0, 1.0], np.float64) / 8.0
    g2 = np.outer(f1, f1) * 4.0
    wf = (weight.astype(np.float64) * (1.0 / np.sqrt(IC * 9)))[:, :, ::-1, ::-1]
    G2 = np.zeros((OC, IC, 6, 6))
    for ky in range(4):
        for kx in range(4):
            G2[:, :, ky:ky + 3, kx:kx + 3] += g2[ky, kx] * wf
    return G2


def _tap_lhsT(G2s: np.ndarray, b: int, dy: int, dx: int) -> np.ndarray:
    """lhsT [ic, a*64+oc] for tap (dy,dx) of group b from modulated G2."""
    out = np.empty((IC, 128), np.float64)
    for a in range(2):
        out[:, a * 64:(a + 1) * 64] = G2s[:, :, 1 - a + 2 * dy, 1 - b + 2 * dx].T
    return out


def _build():
    nc = bacc.Bacc("TRN2", target_bir_lowering=False, debug=False,
                   num_devices=N_CORES)
    x_d = nc.dram_tensor("x", [IC, H, W], BF16, kind="ExternalInput")
    x8_d = nc.dram_tensor("x8", [IC, H + 2, W + 2], F8, kind="ExternalInput")
    wm_d = nc.dram_tensor("wmod", [IC, 2, 5, 128], BF16, kind="ExternalInput")
    w8_d = nc.dram_tensor("w8", [IC, 2, 2, 2, 128], F8, kind="ExternalInput")
    bs_d = nc.dram_tensor("bias_s", [128], mybir.dt.float32, kind="ExternalInput")
    out_d = nc.dram_tensor("out", [OC, 2 * H, 2 * W], BF16,
                           kind="ExternalOutput")

    with tile.TileContext(nc) as tc, ExitStack() as ctx:
        const = ctx.enter_context(tc.tile_pool(name="const", bufs=1))
        outp = ctx.enter_context(tc.tile_pool(name="outp", bufs=4))
        pp = ctx.enter_context(tc.tile_pool(name="pp", bufs=2, space="PSUM"))

        # PE warmup: dep-free matmuls into the ps0 slot, discarded. Ramps the
        # PE p-state (full clock after ~3us continuous) while x/wmod stream in.
        scratch = const.tile([128, 128], BF16)
        nc.vector.memset(scratch, 0)
        ps_w = pp.tile([128, BAND, 128], mybir.dt.float32, tag="ps0", name="ps_w")
        for i in range(N_WARM):
            nc.tensor.matmul(out=ps_w[:, 0:1, :], lhsT=scratch, rhs=scratch,
                             start=(i == 0), stop=(i == N_WARM - 1),
                             skip_group_check=True)

        # weights + x on the FAST rings (sync/scalar); gpsimd ring is slow.
        # q1 (sync) priority order: wmod g0, xb rows 0:10, w8, rest of xb.
        # q10 (scalar): wmod g1 + pre-padded x8 (contiguous per partition).
        wmod = const.tile([IC, 2, 5, 128], BF16)
        w8 = const.tile([IC, 2, 2, 2, 128], F8)
        xt = const.tile([IC, H, W], BF16)
        x8t = const.tile([IC, H + 2, W + 2], F8)
        bias_s = const.tile([128, 1], mybir.dt.float32)

        nc.sync.dma_start(out=wmod[:, 0], in_=wm_d.ap()[:, 0])
        r0, r1 = X_CHUNKS[0]
        nc.sync.dma_start(out=xt[:, r0:r1, :], in_=x_d.ap()[:, r0:r1, :])
        nc.sync.dma_start(out=w8, in_=w8_d.ap())
        for r0, r1 in X_CHUNKS[1:]:
            nc.sync.dma_start(out=xt[:, r0:r1, :], in_=x_d.ap()[:, r0:r1, :])
        nc.scalar.dma_start(out=wmod[:, 1], in_=wm_d.ap()[:, 1])
        for p0, p1 in X8_CHUNKS:
            nc.scalar.dma_start(out=x8t[:, p0:p1, :], in_=x8_d.ap()[:, p0:p1, :])
        nc.gpsimd.dma_start(out=bias_s, in_=bs_d.ap().unsqueeze(1))

        x8_base = x8t[:, :, :]          # AP over the padded fp8 tile

        for i0, bw in BANDS:
            nh = bw // 4
            ob = outp.tile([128, BAND, 256], BF16, tag="ob", name="ob")
            ob_r = ob.rearrange("p r (w two) -> p r two w", two=2)
            for g in range(2):
                ps = pp.tile([128, BAND, 128], mybir.dt.float32, tag=f"ps{g}",
                             name=f"ps{g}")
                # strong taps in bf16, clipped to the valid (unpadded) region
                for ti, (dy, dx) in enumerate(STRONG[g]):
                    c0, c1 = (1, 128) if dx == 0 else (0, 127) if dx == 2 \
                        else (0, 128)
                    for h in range(nh):
                        rb0, rb1 = 4 * h, 4 * h + 4
                        xr0 = i0 + rb0 + dy - 1
                        if xr0 < 0:
                            rb0 += 1
                            xr0 = 0
                        xr1 = xr0 + (rb1 - rb0)
                        if xr1 > H:
                            rb1 -= xr1 - H
                            xr1 = H
                        nc.tensor.matmul(
                            out=ps[:, rb0:rb1, c0:c1],
                            lhsT=wmod[:, g, ti, :],
                            rhs=xt[:, xr0:xr1, c0 + dx - 1:c1 + dx - 1],
                            start=(ti == 0), stop=False,
                        )
                # weak taps: fp8 DoubleRow, 2 taps per pass, padded x copy
                for p, ((dy1, dx1), (dy2, dx2)) in enumerate(PAIRS[g]):
                    kts = (dy2 - dy1) * (W + 2) + (dx2 - dx1)
                    for h in range(nh):
                        off = (i0 + 4 * h + dy1) * (W + 2) + dx1
                        rhs = bass.AP(
                            tensor=x8_base.tensor,
                            offset=x8_base.offset + off,
                            ap=[list(x8_base.ap[0]), [kts, 2],
                                [W + 2, 4], [1, 128]],
                        )
                        nc.tensor.matmul(
                            out=ps[:, 4 * h:4 * h + 4, :],
                            lhsT=w8[:, g, p, :, :], rhs=rhs,
                            start=False, stop=(p == len(PAIRS[g]) - 1),
                            perf_mode=mybir.MatmulPerfMode.DoubleRow,
                        )
                nc.scalar.activation(
                    out=ob_r[:, 0:bw, g, :], in_=ps[:, 0:bw, :],
                    func=mybir.ActivationFunctionType.Prelu,
                    bias=bias_s, scale=SQ2, alpha=ALPHA,
                )
            # partition p = a*64+oc, band row r -> out[oc, 2(i0+r)+a, :]
            h0 = 2 * i0
            for a, eng in ((0, nc.scalar), (1, nc.sync)):
                dst = bass.AP(
                    tensor=out_d, offset=(h0 + a) * (2 * W),
                    ap=[[4 * H * W, OC], [4 * W, bw], [1, 2 * W]],
                )
                eng.dma_start(out=dst, in_=ob[a * 64:(a + 1) * 64, 0:bw])
    nc.compile()
    return nc


def _get_nc():
    global _NC
    if _NC is None:
        _NC = _build()
    return _NC


def _in_maps(x, c, weight, bias, w_affine, b_affine):
    x = np.asarray(x, np.float32)
    c = np.asarray(c, np.float32)
    G2 = _polyphase_g2(np.asarray(weight, np.float32))
    # host-folded hyper modulation: s[n, ic] = 1 + tanh(affine(c))
    y = c @ (np.asarray(w_affine, np.float32) * (1.0 / np.sqrt(CD))).T \
        + np.asarray(b_affine, np.float32)
    s = (1.0 + np.tanh(y)).astype(np.float64)               # [N_CORES, IC]
    bias_s = (SQ2 * np.tile(np.asarray(bias, np.float32), 2)).astype(np.float32)
    maps = []
    for n in range(N_CORES):
        G2s = G2 * s[n][None, :, None, None]
        wm = np.empty((IC, 2, 5, 128), np.float64)
        w8 = np.empty((IC, 2, 2, 2, 128), np.float64)
        for g in range(2):
            for ti, (dy, dx) in enumerate(STRONG[g]):
                wm[:, g, ti] = _tap_lhsT(G2s, g, dy, dx)
            for p, pair in enumerate(PAIRS[g]):
                for j, (dy, dx) in enumerate(pair):
                    w8[:, g, p, j] = _tap_lhsT(G2s, g, dy, dx) * 2.0**K8
        xb = x[n].astype(NP_BF16)
        x8p = np.zeros((IC, H + 2, W + 2), NP_F8)
        x8p[:, 1:H + 1, 1:W + 1] = \
            (xb.astype(np.float32) * 2.0**-K8).astype(NP_F8)
        maps.append({
            "x": xb,
            "x8": x8p,
            "wmod": wm.astype(NP_BF16),
            "w8": w8.astype(NP_F8),
            "bias_s": bias_s,
        })
    return maps


def kernel(x, c, weight, bias, w_affine, b_affine):
    nc = _get_nc()
    res = run_bass_kernel_spmd(
        nc, _in_maps(x, c, weight, bias, w_affine, b_affine),
        core_ids=list(range(N_CORES)))
    return np.stack([np.asarray(res.results[n]["out"], np.float32)
                     for n in range(N_CORES)], axis=0)


def run_traced(x, c, weight, bias, w_affine, b_affine, **trace_kwargs):
    """Like kernel() but returns the full BassKernelResults (for profiling)."""
    nc = _get_nc()
    return run_bass_kernel_spmd(
        nc, _in_maps(x, c, weight, bias, w_affine, b_affine),
        core_ids=list(range(N_CORES)), trace=True, **trace_kwargs)


# revision 15
# speedup vs baseline: 1.0335x; 1.0335x over previous
"""Trainium2 Bass kernel for nn_Conv2dLayer_14998025797815.

Reference op (StyleGAN-style hyper-modulated upsampling conv):
  x [8,128,128,128] fp32 ; c [8,512] ; weight [64,128,3,3] ; bias [64]
  w_affine [128,512] ; b_affine [128]
    y  = c @ (w_affine/sqrt(512)).T + b_affine
    x *= (1 + tanh(y))[:,:,None,None]
    x  = upfirdn2d(x, outer(f,f), up=2, pad=(3,2))        f = [1,3,3,1]/8, gain 4
    x  = conv2d(x, flip(weight)/sqrt(128*9), VALID)       -> [8,64,256,256]
    x  = clip(lrelu(x + bias, 0.2) * sqrt(2), +-256)

Math: the zero-insert upsample + 4x4 FIR + 3x3 conv compose into one 6x6
kernel G2 = conv_full(FIR*4, flip(w)); polyphase decomposition over output
parity (a,b) gives four 3x3 convs on the ORIGINAL 128x128 grid:
    out[n,oc,2i+a,2j+b] = sum_{ic,dy,dx} xpad[n,ic,i+dy,j+dx] * G2[oc,ic,1-a+2dy,1-b+2dx]
The per-sample channel scale s = 1+tanh(affine(c)) is folded into the conv
weights on the host. The +-256 clamp is omitted: |out| <= ~6 here.

Mixed precision: the separable FIR makes tap energies asymmetric. Per
matmul group (column parity b), the per-tap share of output energy is
  dx weak col (3 taps): 0.61% + 2.11% + 0.61%, plus two 6.11% taps at
  dy edges of the other cols; the rest carry >11% each.
The 4 weakest taps per group (9.4% of energy) are computed in fp8 e4m3
with DoubleRow perf mode - 2 taps (K=256) per PE pass at the cost of one
bf16 pass - shrinking 9 passes to 7 (5 bf16 + 2 fp8 pairs). Measured
end-to-end rel err stays ~1e-2 vs the 2e-2 budget (quantization error
scales as sqrt(energy share)). fp8 taps read a zero-PADDED fp8 x copy so
k-tile pairs never need edge clipping; bf16 taps read the unpadded bf16
x and clip edge matmuls to the valid region (clipped elements are
exactly the zero-pad contributions). Weights scale 2^K8 keeps fp8
weights in e4m3's normal range (x scaled 2^-K8 to compensate).

Mapping to 8 NeuronCores: data-parallel over batch (one image per core).
Per core: ic on SBUF partitions; two matmul groups by column parity b,
each M = 128 = (a in {0,1}) x (oc in 0..63); PSUM accumulation per
4-row bank; one Prelu ACT per (band, group) fuses bias + leaky-relu +
sqrt2 gain and writes the b-parity interleave in bf16; output DMA
scatters [p=a*64+oc, i, w] rows into NCHW DRAM on the scalar (a=0) and
sync (a=1) rings (bf16, widened to fp32 on the host).
"""
import sys

if '/opt/trn_rl_repo' not in sys.path:
    sys.path.insert(0, '/opt/trn_rl_repo')

from contextlib import ExitStack

import numpy as np

import concourse.bass as bass
import concourse.tile as tile
from concourse import bacc, mybir
from concourse.bass_utils import run_bass_kernel_spmd

N_CORES = 8
IC, OC, H, W = 128, 64, 128, 128
CD = 512
SQ2 = float(np.sqrt(2.0))
ALPHA = 0.2

BAND = 8                      # output phase-grid rows per band
NB = H // BAND                # 16 bands
N_WARM = 48                   # dep-free matmuls: PE p-state ramp; DMA queues
                              # only start after the ~8us engine preamble, so
                              # warmup also covers the first x/w arrival
K8 = 3                        # fp8 scale split: w*2^K8, x*2^-K8

BF16 = mybir.dt.bfloat16
F8 = mybir.dt.float8e4
NP_BF16 = mybir.dt.np(BF16)
NP_F8 = mybir.dt.np(F8)

X_CHUNKS = [(0, 12), (20, 48), (48, 128)]     # bf16 x load, sync ring (rows)
X_CHUNK_GP = (12, 20)                         # early slice on the gpsimd ring
                                              # (it clears the preamble ~5us
                                              # before sync/scalar do)
X8_CHUNKS = [(0, 20), (20, 130)]              # PRE-PADDED fp8 x load (rows)

# 15 8-row bands + two 4-row bands: a small final band shrinks the tail
# (last ACT + last output DMA are half-size)
BANDS = [(i * BAND, BAND) for i in range(NB - 1)] + \
        [(H - BAND, 4), (H - 4, 4)]

# weak taps per group, as DoubleRow pairs; the rest in bf16. Group 0 gets
# one extra pair (asymmetric): total fp8 energy share ~18.6%, measured
# end-to-end rel err 1.56e-2 vs the 2e-2 budget.
PAIRS = {0: [((0, 2), (2, 2)), ((1, 2), (2, 0)), ((0, 0), (0, 1))],
         1: [((0, 0), (2, 0)), ((1, 0), (2, 2))]}
STRONG = {g: [t for t in [(dy, dx) for dy in range(3) for dx in range(3)]
              if t not in {t_ for pr in PAIRS[g] for t_ in pr}]
          for g in (0, 1)}    # 5 taps per group

_NC = None                    # cached compiled Bass program


def _polyphase_g2(weight: np.ndarray) -> np.ndarray:
    """[64,128,3,3] conv weight -> combined 6x6 polyphase kernel G2 (fp64)."""
    f1 = np.array([1.0, 3.0, 3.0, 1.0], np.float64) / 8.0
    g2 = np.outer(f1, f1) * 4.0
    wf = (weight.astype(np.float64) * (1.0 / np.sqrt(IC * 9)))[:, :, ::-1, ::-1]
    G2 = np.zeros((OC, IC, 6, 6))
    for ky in range(4):
        for kx in range(4):
            G2[:, :, ky:ky + 3, kx:kx + 3] += g2[ky, kx] * wf
    return G2


def _tap_lhsT(G2s: np.ndarray, b: int, dy: int, dx: int) -> np.ndarray:
    """lhsT [ic, a*64+oc] for tap (dy,dx) of group b from modulated G2."""
    out = np.empty((IC, 128), np.float64)
    for a in range(2):
        out[:, a * 64:(a + 1) * 64] = G2s[:, :, 1 - a + 2 * dy, 1 - b + 2 * dx].T
    return out


def _build():
    nc = bacc.Bacc("TRN2", target_bir_lowering=False, debug=False,
                   num_devices=N_CORES)
    x_d = nc.dram_tensor("x", [IC, H, W], BF16, kind="ExternalInput")
    x8_d = nc.dram_tensor("x8", [IC, H + 2, W + 2], F8, kind="ExternalInput")
    wm_d = nc.dram_tensor("wmod", [IC, 2, 5, 128], BF16, kind="ExternalInput")
    w8_d = nc.dram_tensor("w8", [IC, 2, 3, 2, 128], F8, kind="ExternalInput")
    bs_d = nc.dram_tensor("bias_s", [128], mybir.dt.float32, kind="ExternalInput")
    out_d = nc.dram_tensor("out", [OC, 2 * H, 2 * W], BF16,
                           kind="ExternalOutput")

    with tile.TileContext(nc) as tc, ExitStack() as ctx:
        const = ctx.enter_context(tc.tile_pool(name="const", bufs=1))
        outp = ctx.enter_context(tc.tile_pool(name="outp", bufs=4))
        pp = ctx.enter_context(tc.tile_pool(name="pp", bufs=2, space="PSUM"))

        # PE warmup: dep-free matmuls into the ps0 slot, discarded. Ramps the
        # PE p-state (full clock after ~3us continuous) while x/wmod stream in.
        scratch = const.tile([128, 128], BF16)
        nc.vector.memset(scratch, 0)
        ps_w = pp.tile([128, BAND, 128], mybir.dt.float32, tag="ps0", name="ps_w")
        for i in range(N_WARM):
            nc.tensor.matmul(out=ps_w[:, 0:1, :], lhsT=scratch, rhs=scratch,
                             start=(i == 0), stop=(i == N_WARM - 1),
                             skip_group_check=True)

        # weights + x on the FAST rings (sync/scalar); gpsimd ring is slow.
        # q1 (sync) priority order: wmod g0, xb rows 0:10, w8, rest of xb.
        # q10 (scalar): wmod g1 + pre-padded x8 (contiguous per partition).
        wmod = const.tile([IC, 2, 5, 128], BF16)
        w8 = const.tile([IC, 2, 3, 2, 128], F8)
        xt = const.tile([IC, H, W], BF16)
        x8t = const.tile([IC, H + 2, W + 2], F8)
        bias_s = const.tile([128, 1], mybir.dt.float32)

        g0, g1 = X_CHUNK_GP
        nc.gpsimd.dma_start(out=xt[:, g0:g1, :], in_=x_d.ap()[:, g0:g1, :])
        nc.gpsimd.dma_start(out=bias_s, in_=bs_d.ap().unsqueeze(1))
        nc.sync.dma_start(out=wmod[:, 0], in_=wm_d.ap()[:, 0])
        r0, r1 = X_CHUNKS[0]
        nc.sync.dma_start(out=xt[:, r0:r1, :], in_=x_d.ap()[:, r0:r1, :])
        nc.sync.dma_start(out=w8, in_=w8_d.ap())
        for r0, r1 in X_CHUNKS[1:]:
            nc.sync.dma_start(out=xt[:, r0:r1, :], in_=x_d.ap()[:, r0:r1, :])
        nc.scalar.dma_start(out=wmod[:, 1], in_=wm_d.ap()[:, 1])
        for p0, p1 in X8_CHUNKS:
            nc.scalar.dma_start(out=x8t[:, p0:p1, :], in_=x8_d.ap()[:, p0:p1, :])

        x8_base = x8t[:, :, :]          # AP over the padded fp8 tile

        for i0, bw in BANDS:
            nh = bw // 4
            ob = outp.tile([128, BAND, 256], BF16, tag="ob", name="ob")
            ob_r = ob.rearrange("p r (w two) -> p r two w", two=2)
            for g in range(2):
                ps = pp.tile([128, BAND, 128], mybir.dt.float32, tag=f"ps{g}",
                             name=f"ps{g}")
                # strong taps in bf16, clipped to the valid (unpadded) region
                for ti, (dy, dx) in enumerate(STRONG[g]):
                    c0, c1 = (1, 128) if dx == 0 else (0, 127) if dx == 2 \
                        else (0, 128)
                    for h in range(nh):
                        rb0, rb1 = 4 * h, 4 * h + 4
                        xr0 = i0 + rb0 + dy - 1
                        if xr0 < 0:
                            rb0 += 1
                            xr0 = 0
                        xr1 = xr0 + (rb1 - rb0)
                        if xr1 > H:
                            rb1 -= xr1 - H
                            xr1 = H
                        nc.tensor.matmul(
                            out=ps[:, rb0:rb1, c0:c1],
                            lhsT=wmod[:, g, ti, :],
                            rhs=xt[:, xr0:xr1, c0 + dx - 1:c1 + dx - 1],
                            start=(ti == 0), stop=False,
                        )
                # weak taps: fp8 DoubleRow, 2 taps per pass, padded x copy
                for p, ((dy1, dx1), (dy2, dx2)) in enumerate(PAIRS[g]):
                    kts = (dy2 - dy1) * (W + 2) + (dx2 - dx1)
                    for h in range(nh):
                        off = (i0 + 4 * h + dy1) * (W + 2) + dx1
                        rhs = bass.AP(
                            tensor=x8_base.tensor,
                            offset=x8_base.offset + off,
                            ap=[list(x8_base.ap[0]), [kts, 2],
                                [W + 2, 4], [1, 128]],
                        )
                        nc.tensor.matmul(
                            out=ps[:, 4 * h:4 * h + 4, :],
                            lhsT=w8[:, g, p, :, :], rhs=rhs,
                            start=False, stop=(p == len(PAIRS[g]) - 1),
                            perf_mode=mybir.MatmulPerfMode.DoubleRow,
                        )
                nc.scalar.activation(
                    out=ob_r[:, 0:bw, g, :], in_=ps[:, 0:bw, :],
                    func=mybir.ActivationFunctionType.Prelu,
                    bias=bias_s, scale=SQ2, alpha=ALPHA,
                )
            # partition p = a*64+oc, band row r -> out[oc, 2(i0+r)+a, :]
            h0 = 2 * i0
            for a, eng in ((0, nc.scalar), (1, nc.sync)):
                dst = bass.AP(
                    tensor=out_d, offset=(h0 + a) * (2 * W),
                    ap=[[4 * H * W, OC], [4 * W, bw], [1, 2 * W]],
                )
                eng.dma_start(out=dst, in_=ob[a * 64:(a + 1) * 64, 0:bw])
    nc.compile()
    return nc


def _get_nc():
    global _NC
    if _NC is None:
        _NC = _build()
    return _NC


def _in_maps(x, c, weight, bias, w_affine, b_affine):
    x = np.asarray(x, np.float32)
    c = np.asarray(c, np.float32)
    G2 = _polyphase_g2(np.asarray(weight, np.float32))
    # host-folded hyper modulation: s[n, ic] = 1 + tanh(affine(c))
    y = c @ (np.asarray(w_affine, np.float32) * (1.0 / np.sqrt(CD))).T \
        + np.asarray(b_affine, np.float32)
    s = (1.0 + np.tanh(y)).astype(np.float64)               # [N_CORES, IC]
    bias_s = (SQ2 * np.tile(np.asarray(bias, np.float32), 2)).astype(np.float32)
    maps = []
    for n in range(N_CORES):
        G2s = G2 * s[n][None, :, None, None]
        wm = np.zeros((IC, 2, 5, 128), np.float64)
        w8 = np.zeros((IC, 2, 3, 2, 128), np.float64)
        for g in range(2):
            for ti, (dy, dx) in enumerate(STRONG[g]):
                wm[:, g, ti] = _tap_lhsT(G2s, g, dy, dx)
            for p, pair in enumerate(PAIRS[g]):
                for j, (dy, dx) in enumerate(pair):
                    w8[:, g, p, j] = _tap_lhsT(G2s, g, dy, dx) * 2.0**K8
        xb = x[n].astype(NP_BF16)
        x8p = np.zeros((IC, H + 2, W + 2), NP_F8)
        x8p[:, 1:H + 1, 1:W + 1] = \
            (xb.astype(np.float32) * 2.0**-K8).astype(NP_F8)
        maps.append({
            "x": xb,
            "x8": x8p,
            "wmod": wm.astype(NP_BF16),
            "w8": w8.astype(NP_F8),
            "bias_s": bias_s,
        })
    return maps


def kernel(x, c, weight, bias, w_affine, b_affine):
    nc = _get_nc()
    res = run_bass_kernel_spmd(
        nc, _in_maps(x, c, weight, bias, w_affine, b_affine),
        core_ids=list(range(N_CORES)))
    return np.stack([np.asarray(res.results[n]["out"], np.float32)
                     for n in range(N_CORES)], axis=0)


def run_traced(x, c, weight, bias, w_affine, b_affine, **trace_kwargs):
    """Like kernel() but returns the full BassKernelResults (for profiling)."""
    nc = _get_nc()
    return run_bass_kernel_spmd(
        nc, _in_maps(x, c, weight, bias, w_affine, b_affine),
        core_ids=list(range(N_CORES)), trace=True, **trace_kwargs)
